# revision 8
# baseline (speedup 1.0000x reference)
"""MoE gate (DeepSeek-style group-limited routing) Trainium2 Bass kernel.

Full inputs:  x [16384, 7168] f32, weight [256, 7168] f32, bias [256] f32
Outputs:      w [16384, 8] f32, idx [16384, 8] int32

Sharding: x is split along tokens across 8 NeuronCores (2048 tokens each);
weight and bias are replicated.  Each core runs the router GEMM
([2048,7168] @ [7168,256]) plus the full group-limited top-k on-chip.
"""

import os
import sys

sys.path.insert(0, "/opt/trn_rl_repo")

import numpy as np

import concourse.bass as bass
import concourse.mybir as mybir
from concourse.bass import ts
from concourse.masks import make_identity
from concourse.tile import TileContext

T_FULL = 16384
D = 7168
E = 256
G = 8                # expert groups
EPG = E // G         # experts per group (32)
TOPK = 8
ROUTE_SCALE = 2.5
N_CORES = 8
TPC = T_FULL // N_CORES   # tokens per core (2048)
P = 128                   # partitions / tile height
DC = D // P               # d-chunks (56)

F32 = mybir.dt.float32
F32R = mybir.dt.float32r
BF16 = mybir.dt.bfloat16
U32 = mybir.dt.uint32
I32 = mybir.dt.int32
Alu = mybir.AluOpType
Act = mybir.ActivationFunctionType

# GEMM operand mode: "f32" (exact, 4 cyc/row), "f32r" (1 cyc/row at N>=256),
# "bf16" (1 cyc/row, reduced precision).
MODE = os.environ.get("BASS_GATE_MODE", "f32r")


def _split_multi_waits(nc: bass.Bass, max_waits: int = 1) -> None:
    """Walrus codegen in this container rejects instructions carrying more
    than one embedded semaphore wait ("Too many sync wait commands").  Hoist
    excess waits into standalone EventSemaphore instructions placed just
    before the owner on the same engine stream — semantically identical."""
    import bass_rust

    n = 0
    for f in nc.m.functions:
        for b in f.blocks:
            out = []
            for ins in b.instructions:
                si = ins.sync_info
                if si is not None and si.on_wait and len(si.on_wait) > max_waits:
                    waits = list(si.on_wait)
                    keep = waits[-max_waits:] if max_waits else []
                    hoist = waits[: len(waits) - max_waits]
                    for w in hoist:
                        es = mybir.InstEventSemaphore(
                            name=f"WSPLIT-{n}",
                            engine=ins.engine,
                            ins=[],
                            outs=[],
                            sync_info=bass_rust.SyncInfo(on_wait=[w], on_update=[]),
                        )
                        n += 1
                        out.append(es)
                    si.on_wait = keep
                out.append(ins)
            b.instructions = out


def build_gate_kernel(tokens_per_core: int = TPC, mode: str = MODE) -> bass.Bass:
    n_tiles = tokens_per_core // P
    nc = bass.Bass(trn_type="TRN2", name=f"moe_gate_{mode}")

    x = nc.dram_tensor("x", [tokens_per_core, D], F32, kind="ExternalInput")
    wdram = nc.dram_tensor("weight", [E, D], F32, kind="ExternalInput")
    bdram = nc.dram_tensor("bias", [E], F32, kind="ExternalInput")
    wout = nc.dram_tensor("w_out", [tokens_per_core, TOPK], F32, kind="ExternalOutput")
    iout = nc.dram_tensor("idx_out", [tokens_per_core, TOPK], I32, kind="ExternalOutput")

    if mode == "f32":
        mm_cast = lambda ap: ap
    elif mode == "f32r":
        mm_cast = lambda ap: ap.bitcast(F32R)
    elif mode == "bf16":
        mm_cast = lambda ap: ap
    else:
        raise ValueError(mode)
    op_dt = BF16 if mode == "bf16" else F32

    with TileContext(nc) as tc:
        with (
            tc.tile_pool(name="const", bufs=1) as cpool,
            tc.tile_pool(name="wT", bufs=1) as wTpool,
            tc.tile_pool(name="wtmp", bufs=4) as wtmp_pool,
            tc.tile_pool(name="xin", bufs=2) as xpool,
            tc.tile_pool(name="xT", bufs=6) as xTpool,
            tc.tile_pool(name="big", bufs=2) as big,
            tc.tile_pool(name="small", bufs=2) as small,
            tc.tile_pool(name="ps_tr", bufs=4, space="PSUM") as ps_tr,
            tc.tile_pool(name="ps_mm", bufs=2, space="PSUM") as ps_mm,
        ):
            # ---- constants ----
            ident = cpool.tile([P, P], F32)
            make_identity(nc, ident)

            bias10 = cpool.tile([P, E], F32)
            nc.sync.dma_start(
                bias10, bdram[:].unsqueeze(0).to_broadcast([P, E])
            )
            # +10 shift keeps every kept score positive so that masked-out
            # entries (exactly 0) can never enter the top-8.
            nc.vector.tensor_scalar_add(bias10, bias10, 10.0)

            # ---- weight transpose: wT[c] is [d=128, e=256] for d-chunk c ----
            wT = [
                wTpool.tile([P, E], op_dt, tag=f"wT{c}", name=f"wT{c}")
                for c in range(DC)
            ]
            for c in range(DC):
                for h in range(2):
                    wtmp = wtmp_pool.tile([P, P], F32)
                    nc.sync.dma_start(wtmp, wdram[ts(h, P), ts(c, P)])
                    pst = ps_tr.tile([P, P], F32)
                    nc.tensor.transpose(pst, wtmp, ident)
                    eng = nc.scalar if (c + h) % 2 else nc.vector
                    if eng is nc.scalar:
                        eng.copy(out=wT[c][:, ts(h, P)], in_=pst)
                    else:
                        eng.tensor_copy(out=wT[c][:, ts(h, P)], in_=pst)

            # ---- main loop over 128-token tiles ----
            for t in range(n_tiles):
                xt = xpool.tile([P, D], F32)
                # split the 3.6 MB load across 8 DMA queues
                dsplit = D // 8
                for s in range(8):
                    nc.sync.dma_start(
                        xt[:, ts(s, dsplit)],
                        x[ts(t, P), ts(s, dsplit)],
                    )

                lg = ps_mm.tile([P, E], F32)
                # software-pipelined by one chunk: PE issue order is
                # transpose(c+1) before matmul(c) so the PSUM->SBUF copy of
                # chunk c hides under transpose(c+1).
                prev = None
                for c in range(DC):
                    pst = ps_tr.tile([P, P], F32)
                    nc.tensor.transpose(pst, xt[:, ts(c, P)], ident)
                    xTc = xTpool.tile([P, P], op_dt)
                    if c % 2:
                        nc.scalar.copy(out=xTc, in_=pst)
                    else:
                        nc.vector.tensor_copy(out=xTc, in_=pst)
                    if prev is not None:
                        nc.tensor.matmul(
                            lg,
                            mm_cast(prev[:]),
                            mm_cast(wT[c - 1][:]),
                            start=(c == 1),
                            stop=False,
                        )
                    prev = xTc
                nc.tensor.matmul(
                    lg, mm_cast(prev[:]), mm_cast(wT[DC - 1][:]), start=False, stop=True
                )

                # ---- routing top-k (all fp32) ----
                orig = big.tile([P, E], F32, tag="orig")
                nc.scalar.activation(orig, lg, Act.Sigmoid)
                sb10 = big.tile([P, E], F32, tag="sb10")
                nc.vector.tensor_add(sb10, orig, bias10)

                # per-group top-8 (only top-2 needed); group scores
                gtmp = small.tile([P, G * 8], F32, tag="gtmp")
                for g in range(G):
                    nc.vector.max(out=gtmp[:, ts(g, 8)], in_=sb10[:, ts(g, EPG)])
                gtr = gtmp.rearrange("p (g k) -> p g k", k=8)
                gs = small.tile([P, G], F32, tag="gs")
                nc.vector.tensor_add(gs, gtr[:, :, 0], gtr[:, :, 1])
                g8 = small.tile([P, 8], F32, tag="g8")
                nc.vector.max(out=g8, in_=gs)

                # masked = (group_score >= 4th_largest) * sb10
                masked = big.tile([P, E], F32, tag="masked")
                nc.vector.scalar_tensor_tensor(
                    out=masked.rearrange("p (g e) -> p g e", e=EPG),
                    in0=gs.unsqueeze(2).to_broadcast([P, G, EPG]),
                    scalar=g8[:, 3:4],
                    in1=sb10.rearrange("p (g e) -> p g e", e=EPG),
                    op0=Alu.is_ge,
                    op1=Alu.mult,
                )

                v8 = small.tile([P, 8], F32, tag="v8")
                nc.vector.max(out=v8, in_=masked)
                idxu = small.tile([P, 8], U32, tag="idxu")
                nc.vector.max_index(idxu, v8, masked)

                # mark the selected positions, then pull their sigmoid scores
                mr = big.tile([P, E], F32, tag="mr")
                nc.vector.match_replace(
                    out=mr, in_to_replace=v8, in_values=masked, imm_value=-1.0
                )
                osel = big.tile([P, E], F32, tag="osel")
                ssum = small.tile([P, 1], F32, tag="ssum")
                nc.vector.scalar_tensor_tensor(
                    out=osel,
                    in0=mr,
                    scalar=0.0,
                    in1=orig,
                    op0=Alu.is_lt,
                    op1=Alu.mult,
                    accum_out=ssum,
                )
                v2 = small.tile([P, 8], F32, tag="v2")
                nc.vector.max(out=v2, in_=osel)
                i2u = small.tile([P, 8], U32, tag="i2u")
                nc.vector.max_index(i2u, v2, osel)

                i2f = small.tile([P, 8], F32, tag="i2f")
                nc.vector.tensor_copy(i2f, i2u)
                idxf = small.tile([P, 8], F32, tag="idxf")
                nc.vector.tensor_copy(idxf, idxu)
                rec = small.tile([P, 1], F32, tag="rec")
                nc.vector.reciprocal(rec, ssum)

                # reorder v2 (sorted by sigmoid) into selection order (idx)
                wraw = small.tile([P, 8], F32, tag="wraw")
                mjv = small.tile([P, 8], F32, tag="mjv")
                for j in range(TOPK):
                    nc.vector.scalar_tensor_tensor(
                        out=mjv,
                        in0=i2f,
                        scalar=idxf[:, j : j + 1],
                        in1=v2,
                        op0=Alu.is_equal,
                        op1=Alu.mult,
                        accum_out=wraw[:, j : j + 1],
                    )
                wfin = small.tile([P, 8], F32, tag="wfin")
                nc.vector.tensor_scalar(
                    out=wfin,
                    in0=wraw,
                    scalar1=rec[:, :1],
                    scalar2=float(ROUTE_SCALE),
                    op0=Alu.mult,
                    op1=Alu.mult,
                )
                idxi = small.tile([P, 8], I32, tag="idxi")
                nc.vector.tensor_copy(idxi, idxu)

                nc.sync.dma_start(wout[ts(t, P), :], wfin)
                nc.sync.dma_start(iout[ts(t, P), :], idxi)

    nc.finalize()
    return nc


_NC_CACHE: dict = {}
last_results = None  # BassKernelResults of the most recent kernel() call


def kernel(x: np.ndarray, weight: np.ndarray, bias: np.ndarray):
    global last_results
    from concourse.bass_utils import run_bass_kernel_spmd

    key = MODE
    if key not in _NC_CACHE:
        nc_new = build_gate_kernel(TPC, MODE)
        _split_multi_waits(nc_new)  # required by this walrus build (HW path only)
        _NC_CACHE[key] = nc_new
    nc = _NC_CACHE[key]

    x = np.ascontiguousarray(np.asarray(x, dtype=np.float32))
    weight = np.ascontiguousarray(np.asarray(weight, dtype=np.float32))
    bias = np.ascontiguousarray(np.asarray(bias, dtype=np.float32))

    xs = x.reshape(N_CORES, TPC, D)
    in_maps = [
        {"x": np.ascontiguousarray(xs[i]), "weight": weight, "bias": bias}
        for i in range(N_CORES)
    ]
    trace = bool(int(os.environ.get("BASS_GATE_TRACE", "0")))
    res = run_bass_kernel_spmd(
        nc, in_maps, core_ids=list(range(N_CORES)), trace=trace
    )
    last_results = res
    w = np.concatenate([r["w_out"] for r in res.results], axis=0)
    idx = np.concatenate([r["idx_out"] for r in res.results], axis=0)
    return w.astype(np.float32), idx.astype(np.int32)
